# revision 1
# baseline (speedup 1.0000x reference)
"""Trainium2 Bass kernel for MiMoAudio attention (GQA + neox RoPE + causal softmax + o_proj).

Strategy (tensor-parallel over heads, 8 cores):
  - Each core owns 2 of the 16 q heads (128 q channels) and the single kv head
    (64 channels) that those q heads attend to (GQA group).
  - Host pre-transposes hidden_states to xT [H, B*S] so every on-device matmul
    contracts over the partition dim with no on-device transposition of x.
  - All activations live "feature-on-partitions" (transposed domain):
      qT [128, T], kT/vT in kvT [128, T], scoresT [j, i], attnT [d', i].
    Softmax runs without max-subtraction (logits are O(10), fp32-exp safe);
    the denominator is obtained by an appended ones-row in the PV matmul.
  - o_proj row-slice per core produces a partial [T, H] output; host sums the
    8 partials (the TP all-reduce, done at unshard time).
"""

import os
import numpy as np

# Problem constants (hardcoded per contract; kernel.py must be self-contained).
B = 2
S = 2048
T = B * S          # 4096 flattened tokens
H = 1024           # hidden
HD = 64            # head dim
P = 128
NCORES = 8
THETA = 10000.0
SCALE = HD ** -0.5
NBLK = T // 512    # 8 token blocks of 512
HO = H // P        # 8 hidden chunks of 128
SJT = S // P       # 16 key tiles per batch

_NC_CACHE = {}
LAST_RESULT = None  # stash of the last BassKernelResults (for test harnesses)


def _ensure_ntff_hook():
    """Provide antenv.axon_hooks if the image lacks it, so BASS_TRACE=1
    profiling works under axon instead of crashing on import."""
    import sys
    import types

    try:
        import antenv.axon_hooks  # noqa: F401
        return
    except ImportError:
        pass
    mod = types.ModuleType("antenv.axon_hooks")
    mod._hook = None

    def set_axon_ntff_profile_hook(h):
        mod._hook = h

    def get_axon_ntff_profile_hook():
        return mod._hook

    mod.set_axon_ntff_profile_hook = set_axon_ntff_profile_hook
    mod.get_axon_ntff_profile_hook = get_axon_ntff_profile_hook
    sys.modules["antenv.axon_hooks"] = mod
    try:
        import antenv

        antenv.axon_hooks = mod
    except ImportError:
        pass
    try:
        from trn_agent_boot.trn_boot import _ntff_profile_via_ctypes

        hook = _ntff_profile_via_ctypes("/opt/axon/libaxon_pjrt.so")
        if hook is not None:
            mod.set_axon_ntff_profile_hook(hook)
    except Exception:
        pass


_ensure_ntff_hook()


def _build_nc(mm_mode="f32r"):
    import concourse.bass as bass
    import concourse.mybir as mybir
    import concourse.tile as tile
    from concourse import bacc

    f32 = mybir.dt.float32
    Act = mybir.ActivationFunctionType

    # Matmul-operand dtype: float32r (full fp32 bits in memory; PE runs the
    # fast reduced-precision multiply path at 1 cycle/row instead of 4).
    # Walrus requires every producer feeding an FP32r matmul to emit an
    # FP32r-typed output, so all matmul-feeding tensors use `adt` natively.
    if mm_mode == "f32r":
        adt = mybir.dt.float32r
    elif mm_mode == "f32":
        adt = f32
    elif mm_mode == "bf16":
        adt = mybir.dt.bfloat16
    else:
        raise ValueError(mm_mode)
    two_byte = adt == mybir.dt.bfloat16

    nc = bacc.Bacc(None, target_bir_lowering=False, debug=False)

    # --- DRAM I/O ------------------------------------------------------------
    xT_d = nc.dram_tensor("xT", [H, T], adt, kind="ExternalInput")
    wq_d = nc.dram_tensor("wq", [H, P], adt, kind="ExternalInput")
    bq_d = nc.dram_tensor("bq", [P, 1], f32, kind="ExternalInput")
    wkv_d = nc.dram_tensor("wkv", [H, P], adt, kind="ExternalInput")
    bkv_d = nc.dram_tensor("bkv", [P, 1], f32, kind="ExternalInput")
    wo_d = nc.dram_tensor("wo", [P, H], adt, kind="ExternalInput")
    cos_d = nc.dram_tensor("cosT", [P, S], f32, kind="ExternalInput")
    sin_d = nc.dram_tensor("sinT", [P, S], f32, kind="ExternalInput")
    perm_d = nc.dram_tensor("perm", [P, P], adt, kind="ExternalInput")
    id_d = nc.dram_tensor("ident", [P, P], adt, kind="ExternalInput")
    ones_d = nc.dram_tensor("ones64", [1, 64], adt, kind="ExternalInput")
    onescol_d = nc.dram_tensor("onescol", [P, 2 * SJT], adt, kind="ExternalInput")
    out_d = nc.dram_tensor("out", [T, H], f32, kind="ExternalOutput")

    with tile.TileContext(nc) as tc:
        with (
            tc.tile_pool(name="const", bufs=1) as cpool,
            tc.tile_pool(name="persist", bufs=1) as ppool,
            tc.tile_pool(name="xt", bufs=2) as xt_pool,
            tc.tile_pool(name="ptile", bufs=4) as p_pool,
            tc.tile_pool(name="attn", bufs=2) as attn_pool,
            tc.tile_pool(name="attn1", bufs=2) as a1_pool,
            tc.tile_pool(name="recip", bufs=2) as r_pool,
            tc.tile_pool(name="tmp", bufs=4) as tmp_pool,
            tc.tile_pool(name="osb", bufs=3) as osb_pool,
            tc.tile_pool(name="mm_ps", bufs=2, space="PSUM") as mm_ps,
            tc.tile_pool(name="s_ps", bufs=3, space="PSUM") as s_ps,
            tc.tile_pool(name="o_ps", bufs=2, space="PSUM") as o_ps,
            tc.tile_pool(name="b_ps", bufs=1, space="PSUM") as b_ps,
        ):
            # --- constant loads ---------------------------------------------
            wq_sb = cpool.tile([P, HO, P], adt)
            nc.sync.dma_start(wq_sb[:], wq_d[:].rearrange("(o p) m -> p o m", p=P))
            wkv_sb = cpool.tile([P, HO, P], adt)
            nc.sync.dma_start(wkv_sb[:], wkv_d[:].rearrange("(o p) m -> p o m", p=P))
            bq_sb = cpool.tile([P, 1], f32)
            nc.sync.dma_start(bq_sb[:], bq_d[:])
            bkv_sb = cpool.tile([P, 1], f32)
            nc.sync.dma_start(bkv_sb[:], bkv_d[:])
            wo_sb = cpool.tile([P, H], adt)
            nc.sync.dma_start(wo_sb[:], wo_d[:])
            cos_sb = cpool.tile([P, S], f32)
            nc.sync.dma_start(cos_sb[:], cos_d[:])
            sin_sb = cpool.tile([P, S], f32)
            nc.sync.dma_start(sin_sb[:], sin_d[:])
            perm_sb = cpool.tile([P, P], adt)
            nc.sync.dma_start(perm_sb[:], perm_d[:])
            id_sb = cpool.tile([P, P], adt)
            nc.sync.dma_start(id_sb[:], id_d[:])
            ones_sb = cpool.tile([1, 64], adt)
            nc.sync.dma_start(ones_sb[:], ones_d[:])

            # --- persistent activation tiles --------------------------------
            qT = ppool.tile([P, T], adt)        # 2 q heads stacked (rows h*64+d)
            kvT = ppool.tile([P, T], adt)       # k rows 0:64, v rows 64:128
            khi = ppool.tile([P, T], adt)       # k duplicated at rows 64:128
            vnat = ppool.tile([P, 2 * SJT, 72], adt)  # v natural [j, d] per tile + ones col

            # ones column for the PV denominator row (memset can't emit f32r)
            nc.sync.dma_start(
                vnat[:, :, 64:65], onescol_d[:].rearrange("p (j o) -> p j o", o=1)
            )

            xT_r = xT_d[:].rearrange("(o p) t -> p o t", p=P)

            # --- phase A: QKV projection + RoPE + layout prep ---------------
            for blk in range(NBLK):
                tb = slice(blk * 512, (blk + 1) * 512)
                xt = xt_pool.tile([P, HO, 512], adt)
                nc.sync.dma_start(xt[:], xT_r[:, :, tb])

                q_ps = mm_ps.tile([P, 512], f32, tag="mmps")
                for o in range(HO):
                    nc.tensor.matmul(
                        q_ps[:], wq_sb[:, o, :], xt[:, o, :],
                        start=(o == 0), stop=(o == HO - 1),
                    )
                nc.scalar.activation(qT[:, tb], q_ps[:], Act.Identity, bias=bq_sb[:])

                kv_ps = mm_ps.tile([P, 512], f32, tag="mmps")
                for o in range(HO):
                    nc.tensor.matmul(
                        kv_ps[:], wkv_sb[:, o, :], xt[:, o, :],
                        start=(o == 0), stop=(o == HO - 1),
                    )
                nc.scalar.activation(kvT[:, tb], kv_ps[:], Act.Identity, bias=bkv_sb[:])

                sc = (blk * 512) % S
                ss = slice(sc, sc + 512)

                # RoPE(q): partner = perm @ qT (partition rotate by 32 within 64)
                pr = s_ps.tile([P, 512], f32, tag="sps")
                nc.tensor.matmul(pr[:], perm_sb[:], qT[:, tb], start=True, stop=True)
                tmp = tmp_pool.tile([P, 512], f32, tag="ropetmp")
                nc.vector.tensor_mul(tmp[:], pr[:], sin_sb[:, ss])
                nc.vector.tensor_mul(qT[:, tb], qT[:, tb], cos_sb[:, ss])
                nc.vector.tensor_add(qT[:, tb], qT[:, tb], tmp[:])

                # RoPE(k): rows 0:64 only
                prk = s_ps.tile([P, 512], f32, tag="sps")
                nc.tensor.matmul(
                    prk[0:64, :], perm_sb[0:64, 0:64], kvT[0:64, tb],
                    start=True, stop=True,
                )
                tmpk = tmp_pool.tile([P, 512], f32, tag="ropetmp")
                nc.vector.tensor_mul(tmpk[0:64, :], prk[0:64, :], sin_sb[0:64, ss])
                nc.vector.tensor_mul(kvT[0:64, tb], kvT[0:64, tb], cos_sb[0:64, ss])
                nc.vector.tensor_add(kvT[0:64, tb], kvT[0:64, tb], tmpk[0:64, :])

                # duplicate rope'd k at rows 64:128 (for head-1 row-group matmuls)
                nc.sync.dma_start(khi[64:128, tb], kvT[0:64, tb])

                # v natural layout [j, d] per 128-key tile (transpose via PE)
                for jj in range(4):
                    jt = blk * 4 + jj
                    if False:
                        nc.sync.dma_start(
                            vnat[:, jt, 0:64],
                            kvT[64:128, jt * P:(jt + 1) * P],
                            transpose=True,
                        )
                    else:
                        tp = mm_ps.tile([P, 512], adt, tag="mmps")
                        nc.tensor.transpose(
                            tp[:, 0:64], kvT[64:128, jt * P:(jt + 1) * P], id_sb[64:128, 64:128]
                        )
                        nc.vector.tensor_copy(vnat[:, jt, 0:64], tp[:, 0:64])

            # --- phase B: attention + o_proj per (batch, 512-query block) ---
            for b in range(B):
                for ib in range(4):
                    qs = slice(b * S + ib * 512, b * S + (ib + 1) * 512)
                    njt = 4 * (ib + 1)
                    po0 = o_ps.tile([65, 512], f32, tag="ops")
                    po1 = o_ps.tile([65, 512], f32, tag="ops")
                    for j in range(njt):
                        jt = b * SJT + j
                        js = slice(jt * P, (jt + 1) * P)
                        ps0 = s_ps.tile([P, 512], f32, tag="sps")
                        nc.tensor.matmul(
                            ps0[:], kvT[0:64, js], qT[0:64, qs],
                            start=True, stop=True,
                        )
                        ps1 = s_ps.tile([P, 512], f32, tag="sps")
                        nc.tensor.matmul(
                            ps1[:], khi[64:128, js], qT[64:128, qs],
                            start=True, stop=True,
                        )
                        p0 = p_pool.tile([P, 512], adt, tag="ptile")
                        nc.scalar.activation(p0[:], ps0[:], Act.Exp, scale=SCALE)
                        p1 = p_pool.tile([P, 512], adt, tag="ptile")
                        nc.scalar.activation(p1[:], ps1[:], Act.Exp, scale=SCALE)
                        if j >= 4 * ib:
                            # zero entries with key j_g > query i_g on the diagonal band:
                            # keep when  -p + f + (512*ib - 128*j) >= 0
                            base = 512 * ib - 128 * j
                            for pt in (p0, p1):
                                nc.gpsimd.affine_select(
                                    pt[:], pt[:],
                                    pattern=[[1, 512]],
                                    compare_op=mybir.AluOpType.is_ge,
                                    fill=0.0,
                                    base=base,
                                    channel_multiplier=-1,
                                )
                        nc.tensor.matmul(
                            po0[:], vnat[:, jt, 0:65], p0[:],
                            start=(j == 0), stop=(j == njt - 1),
                        )
                        nc.tensor.matmul(
                            po1[:], vnat[:, jt, 0:65], p1[:],
                            start=(j == 0), stop=(j == njt - 1),
                        )
                    # normalize: recip of ones-row, broadcast over 64 partitions via K=1 matmul
                    r0 = r_pool.tile([1, 512], adt, tag="recip")
                    r1 = r_pool.tile([1, 512], adt, tag="recip")
                    with nc.allow_low_precision(reason="f32r keeps fp32 bits"):
                        nc.vector.reciprocal(r0[:], po0[64:65, :])
                        nc.vector.reciprocal(r1[:], po1[64:65, :])
                    pb0 = b_ps.tile([64, 512], f32, tag="bps")
                    nc.tensor.matmul(pb0[:], ones_sb[:], r0[:], start=True, stop=True)
                    pb1 = b_ps.tile([64, 512], f32, tag="bps")
                    nc.tensor.matmul(pb1[:], ones_sb[:], r1[:], start=True, stop=True)
                    pbs0 = tmp_pool.tile([64, 512], f32, tag="pbs")
                    nc.vector.tensor_copy(pbs0[:], pb0[:])
                    pbs1 = tmp_pool.tile([64, 512], f32, tag="pbs")
                    nc.vector.tensor_copy(pbs1[:], pb1[:])
                    at = attn_pool.tile([P, 512], adt, tag="attn")
                    nc.vector.tensor_mul(at[0:64, :], po0[0:64, :], pbs0[:])
                    a1 = a1_pool.tile([64, 512], adt, tag="attn1")
                    nc.vector.tensor_mul(a1[:], po1[0:64, :], pbs1[:])
                    nc.sync.dma_start(at[64:128, :], a1[:])

                    # o_proj for this query block
                    for st in range(4):
                        rows = slice(b * S + ib * 512 + st * P, b * S + ib * 512 + (st + 1) * P)
                        for half in range(2):
                            cols = slice(half * 512, (half + 1) * 512)
                            w_ps = mm_ps.tile([P, 512], f32, tag="mmps")
                            nc.tensor.matmul(
                                w_ps[:], at[:, st * P:(st + 1) * P], wo_sb[:, cols],
                                start=True, stop=True,
                            )
                            osb = osb_pool.tile([P, 512], f32, tag="osb")
                            nc.vector.tensor_copy(osb[:], w_ps[:])
                            nc.sync.dma_start(out_d[rows, cols], osb[:])

    nc.compile()
    return nc


def _get_nc(mm_mode="f32r"):
    if mm_mode not in _NC_CACHE:
        _NC_CACHE[mm_mode] = _build_nc(mm_mode)
    return _NC_CACHE[mm_mode]


def make_in_maps(inputs, mm_mode="f32r"):
    """Host-side sharding/layout prep: returns the 8 per-core input dicts."""
    if mm_mode == "bf16":
        import ml_dtypes

        a_np = ml_dtypes.bfloat16
    else:
        a_np = np.float32
    hidden = np.asarray(inputs["hidden_states"], dtype=np.float32)
    pos = np.asarray(inputs["positions"])
    Wq = np.asarray(inputs["Wq"], dtype=np.float32)
    bq = np.asarray(inputs["bq"], dtype=np.float32)
    Wk = np.asarray(inputs["Wk"], dtype=np.float32)
    bk = np.asarray(inputs["bk"], dtype=np.float32)
    Wv = np.asarray(inputs["Wv"], dtype=np.float32)
    bv = np.asarray(inputs["bv"], dtype=np.float32)
    Wo = np.asarray(inputs["Wo"], dtype=np.float32)

    xT = np.ascontiguousarray(hidden.reshape(T, H).T)

    half = HD // 2  # 32
    inv = 1.0 / THETA ** (np.arange(half, dtype=np.float64) * 2.0 / HD)
    f = pos.astype(np.float64)[None, :] * inv[:, None]          # [32, S]
    cos32 = np.cos(f)
    sin32 = np.sin(f)
    pidx = np.arange(P) % half
    sgn = np.where(np.arange(P) % HD < half, -1.0, 1.0)
    cosT = np.ascontiguousarray(cos32[pidx].astype(np.float32))
    sinT = np.ascontiguousarray((sin32[pidx] * sgn[:, None]).astype(np.float32))

    m = np.arange(P)
    sig = np.where(m % HD < half, m + half, m - half)
    perm = np.zeros((P, P), np.float32)
    perm[sig, m] = 1.0
    ident = np.eye(P, dtype=np.float32)
    ones64 = np.ones((1, 64), np.float32)
    onescol = np.ones((P, 2 * (S // P)), np.float32)

    xTa = xT.astype(a_np)
    in_maps = []
    for c in range(NCORES):
        g = c // 2  # kv head for this core's 2 q heads
        wkv = np.ascontiguousarray(
            np.concatenate(
                [Wk[:, g * HD:(g + 1) * HD], Wv[:, g * HD:(g + 1) * HD]], axis=1
            )
        )
        bkv = np.ascontiguousarray(
            np.concatenate([bk[g * HD:(g + 1) * HD], bv[g * HD:(g + 1) * HD]])[:, None]
        )
        in_maps.append({
            "xT": xTa,
            "wq": np.ascontiguousarray(Wq[:, c * P:(c + 1) * P]).astype(a_np),
            "bq": np.ascontiguousarray(bq[c * P:(c + 1) * P][:, None]),
            "wkv": wkv.astype(a_np),
            "bkv": bkv,
            "wo": np.ascontiguousarray(Wo[c * P:(c + 1) * P, :]).astype(a_np),
            "cosT": cosT,
            "sinT": sinT,
            "perm": perm.astype(a_np),
            "ident": ident.astype(a_np),
            "ones64": ones64.astype(a_np),
            "onescol": onescol.astype(a_np),
        })
    return in_maps


def kernel(**inputs):
    global LAST_RESULT
    from concourse.bass_utils import run_bass_kernel_spmd

    mm_mode = os.environ.get("KERNEL_MM_MODE", "bf16")
    nc = _get_nc(mm_mode)
    in_maps = make_in_maps(inputs, mm_mode)
    res = run_bass_kernel_spmd(nc, in_maps, core_ids=list(range(NCORES)))
    LAST_RESULT = res
    out = res.results[0]["out"].astype(np.float32, copy=True)
    for rr in res.results[1:]:
        out += rr["out"]
    return out.reshape(B, S, H)



# revision 4
# speedup vs baseline: 1.2759x; 1.2759x over previous
"""Trainium2 Bass kernel for MiMoAudio attention (GQA + neox RoPE + causal softmax + o_proj).

Strategy (tensor-parallel over heads, 8 cores):
  - Each core owns 2 of the 16 q heads (128 q channels) and the single kv head
    (64 channels) that those q heads attend to (GQA group).
  - Host pre-transposes hidden_states to xT [H, B*S] so every on-device matmul
    contracts over the partition dim with no on-device transposition of x.
  - All activations live "feature-on-partitions" (transposed domain):
      qT [128, T], kT/vT in kvT [128, T], scoresT [j, i], attnT [d', i].
    Softmax runs without max-subtraction (logits are O(10), fp32-exp safe);
    the denominator is obtained by an appended ones-row in the PV matmul.
  - o_proj row-slice per core produces a partial [T, H] output in bf16; host
    sums the 8 partials in fp32 (the TP all-reduce, done at unshard time).

Perf structure (vs the naive version):
  - Scores for both heads of a j-tile go into one 2-bank PSUM tile [128,1024]
    so a single Exp activation covers both heads (halves ACT instr overhead).
  - On diagonal-band j-tiles, the columns that causality fully masks are
    skipped in the scores matmul, the exp, and the PV matmul (column lo=128*jj
    onward only) — saves ~15% of exp volume and ~10% of attention PE work.
  - Residual staircase masking via a multiply with precomputed bf16 mask
    tiles, split between DVE and gpsimd (gpsimd cannot touch PSUM, so it only
    gets SBUF->SBUF work).
  - o_proj of block N is emitted after the scores of block N+1 (one-stage
    software pipeline) so the PE never waits on the normalize chain.
  - v-transposes are deferred to the end of phase A so the PE stream never
    stalls on the rope chain.
  - Output written as bf16 (halves output DMA); host sums in fp32.
"""

import os
import numpy as np

# Problem constants (hardcoded per contract; kernel.py must be self-contained).
B = 2
S = 2048
T = B * S          # 4096 flattened tokens
H = 1024           # hidden
HD = 64            # head dim
P = 128
NCORES = 8
THETA = 10000.0
SCALE = HD ** -0.5
NBLK = T // 512    # 8 token blocks of 512
HO = H // P        # 8 hidden chunks of 128
SJT = S // P       # 16 key tiles per batch

_NC_CACHE = {}
LAST_RESULT = None  # stash of the last BassKernelResults (for test harnesses)


def _ensure_ntff_hook():
    """Provide antenv.axon_hooks if the image lacks it, so BASS_TRACE=1
    profiling works under axon instead of crashing on import."""
    import sys
    import types

    try:
        import antenv.axon_hooks  # noqa: F401
        return
    except ImportError:
        pass
    mod = types.ModuleType("antenv.axon_hooks")
    mod._hook = None

    def set_axon_ntff_profile_hook(h):
        mod._hook = h

    def get_axon_ntff_profile_hook():
        return mod._hook

    mod.set_axon_ntff_profile_hook = set_axon_ntff_profile_hook
    mod.get_axon_ntff_profile_hook = get_axon_ntff_profile_hook
    sys.modules["antenv.axon_hooks"] = mod
    try:
        import antenv

        antenv.axon_hooks = mod
    except ImportError:
        pass
    try:
        from trn_agent_boot.trn_boot import _ntff_profile_via_ctypes

        hook = _ntff_profile_via_ctypes("/opt/axon/libaxon_pjrt.so")
        if hook is not None:
            mod.set_axon_ntff_profile_hook(hook)
    except Exception:
        pass


_ensure_ntff_hook()


def _build_nc(mm_mode="bf16"):
    import concourse.bass as bass
    import concourse.mybir as mybir
    import concourse.tile as tile
    from concourse import bacc

    assert mm_mode == "bf16", "only bf16 supported"
    f32 = mybir.dt.float32
    adt = mybir.dt.bfloat16
    Act = mybir.ActivationFunctionType

    nc = bacc.Bacc(None, target_bir_lowering=False, debug=False)

    # --- DRAM I/O ------------------------------------------------------------
    xT_d = nc.dram_tensor("xT", [H, T], adt, kind="ExternalInput")
    wq_d = nc.dram_tensor("wq", [H, P], adt, kind="ExternalInput")
    bq_d = nc.dram_tensor("bq", [P, 1], f32, kind="ExternalInput")
    wkv_d = nc.dram_tensor("wkv", [H, P], adt, kind="ExternalInput")
    bkv_d = nc.dram_tensor("bkv", [P, 1], f32, kind="ExternalInput")
    wo_d = nc.dram_tensor("wo", [P, H], adt, kind="ExternalInput")
    cos_d = nc.dram_tensor("cosT", [P, S], adt, kind="ExternalInput")
    sin_d = nc.dram_tensor("sinT", [P, S], adt, kind="ExternalInput")
    perm_d = nc.dram_tensor("perm", [P, P], adt, kind="ExternalInput")
    id_d = nc.dram_tensor("ident", [P, P], adt, kind="ExternalInput")
    ones_d = nc.dram_tensor("ones64", [1, 64], adt, kind="ExternalInput")
    onescol_d = nc.dram_tensor("onescol", [P, 2 * SJT], adt, kind="ExternalInput")
    mask_d = nc.dram_tensor("masks", [P, 4, 2, 512], adt, kind="ExternalInput")
    out_d = nc.dram_tensor("out", [T, H], adt, kind="ExternalOutput")

    with tile.TileContext(nc) as tc:
        with (
            tc.tile_pool(name="const", bufs=1) as cpool,
            tc.tile_pool(name="persist", bufs=1) as ppool,
            tc.tile_pool(name="xt", bufs=2) as xt_pool,
            tc.tile_pool(name="ptile", bufs=3) as p_pool,
            tc.tile_pool(name="attn", bufs=2) as attn_pool,
            tc.tile_pool(name="attn1", bufs=2) as a1_pool,
            tc.tile_pool(name="recip", bufs=4) as r_pool,
            tc.tile_pool(name="tmp", bufs=4) as tmp_pool,
            tc.tile_pool(name="osb", bufs=2) as osb_pool,
            tc.tile_pool(name="sps", bufs=2, space="PSUM") as s_ps,
            tc.tile_pool(name="ops", bufs=2, space="PSUM") as o_ps,
            tc.tile_pool(name="wps", bufs=2, space="PSUM") as w_ps,
        ):
            # --- constant loads ---------------------------------------------
            wq_sb = cpool.tile([P, HO, P], adt)
            nc.sync.dma_start(wq_sb[:], wq_d[:].rearrange("(o p) m -> p o m", p=P))
            wkv_sb = cpool.tile([P, HO, P], adt)
            nc.sync.dma_start(wkv_sb[:], wkv_d[:].rearrange("(o p) m -> p o m", p=P))
            bq_sb = cpool.tile([P, 1], f32)
            nc.sync.dma_start(bq_sb[:], bq_d[:])
            bkv_sb = cpool.tile([P, 1], f32)
            nc.sync.dma_start(bkv_sb[:], bkv_d[:])
            cos_sb = cpool.tile([P, S], adt)
            nc.sync.dma_start(cos_sb[:], cos_d[:])
            sin_sb = cpool.tile([P, S], adt)
            nc.sync.dma_start(sin_sb[:], sin_d[:])
            perm_sb = cpool.tile([P, P], adt)
            nc.sync.dma_start(perm_sb[:], perm_d[:])
            id_sb = cpool.tile([P, P], adt)
            nc.sync.dma_start(id_sb[:], id_d[:])
            ones_sb = cpool.tile([1, 64], adt)
            nc.sync.dma_start(ones_sb[:], ones_d[:])
            wo_sb = cpool.tile([P, H], adt)
            nc.sync.dma_start(wo_sb[:], wo_d[:])
            mask_sb = cpool.tile([P, 4, 2, 512], adt)
            nc.sync.dma_start(mask_sb[:], mask_d[:])

            # --- persistent activation tiles --------------------------------
            qT = ppool.tile([P, T], adt)        # 2 q heads stacked (rows h*64+d)
            kvT = ppool.tile([P, T], adt)       # k rows 0:64, v rows 64:128
            khi = ppool.tile([P, T], adt)       # k duplicated at rows 64:128
            vnat = ppool.tile([P, 2 * SJT, 72], adt)  # v natural [j, d] + ones col

            # ones column for the PV denominator row
            nc.sync.dma_start(
                vnat[:, :, 64:65], onescol_d[:].rearrange("p (j o) -> p j o", o=1)
            )

            xT_r = xT_d[:].rearrange("(o p) t -> p o t", p=P)

            # --- phase A: QKV projection + RoPE -----------------------------
            for blk in range(NBLK):
                tb = slice(blk * 512, (blk + 1) * 512)
                xt = xt_pool.tile([P, HO, 512], adt, tag="xt")
                nc.sync.dma_start(xt[:], xT_r[:, :, tb])

                qkv = s_ps.tile([P, 1024], f32, tag="sps")
                for o in range(HO):
                    nc.tensor.matmul(
                        qkv[:, 0:512], wq_sb[:, o, :], xt[:, o, :],
                        start=(o == 0), stop=(o == HO - 1),
                    )
                for o in range(HO):
                    nc.tensor.matmul(
                        qkv[:, 512:1024], wkv_sb[:, o, :], xt[:, o, :],
                        start=(o == 0), stop=(o == HO - 1),
                    )
                nc.vector.tensor_scalar_add(qT[:, tb], qkv[:, 0:512], bq_sb[:])
                nc.scalar.activation(
                    kvT[:, tb], qkv[:, 512:1024], Act.Identity, bias=bkv_sb[:]
                )

                sc = (blk * 512) % S
                ss = slice(sc, sc + 512)

                # RoPE(q): partner = perm @ qT (partition rotate by 32 within 64)
                pr = w_ps.tile([P, 512], f32, tag="wps")
                nc.tensor.matmul(pr[:], perm_sb[:], qT[:, tb], start=True, stop=True)
                tmp = tmp_pool.tile([P, 512], adt, tag="ropetmp")
                nc.vector.tensor_mul(tmp[:], pr[:], sin_sb[:, ss])
                nc.vector.tensor_mul(qT[:, tb], qT[:, tb], cos_sb[:, ss])
                nc.vector.tensor_add(qT[:, tb], qT[:, tb], tmp[:])

                # RoPE(k): rows 0:64 only
                prk = w_ps.tile([P, 512], f32, tag="wps")
                nc.tensor.matmul(
                    prk[0:64, :], perm_sb[0:64, 0:64], kvT[0:64, tb],
                    start=True, stop=True,
                )
                tmpk = tmp_pool.tile([P, 512], adt, tag="ropetmp")
                nc.vector.tensor_mul(tmpk[0:64, :], prk[0:64, :], sin_sb[0:64, ss])
                nc.vector.tensor_mul(kvT[0:64, tb], kvT[0:64, tb], cos_sb[0:64, ss])
                nc.vector.tensor_add(kvT[0:64, tb], kvT[0:64, tb], tmpk[0:64, :])

                # duplicate rope'd k at rows 64:128 (for head-1 row-group matmuls)
                nc.sync.dma_start(khi[64:128, tb], kvT[0:64, tb])

            # --- phase A2: v natural layout [j, d] per 128-key tile ---------
            for jt in range(2 * SJT):
                tp = w_ps.tile([P, 512], adt, tag="wps")
                nc.tensor.transpose(
                    tp[:, 0:64], kvT[64:128, jt * P:(jt + 1) * P],
                    id_sb[64:128, 64:128],
                )
                if jt % 2 == 0:
                    nc.vector.tensor_copy(vnat[:, jt, 0:64], tp[:, 0:64])
                else:
                    nc.scalar.activation(vnat[:, jt, 0:64], tp[:, 0:64], Act.Copy)

            # --- phase B: attention + (deferred) o_proj ---------------------
            def emit_oproj(prev):
                at_p, b_p, ib_p = prev
                osb = osb_pool.tile([P, 4, H], adt, tag="osb")
                for st in range(4):
                    for half in range(2):
                        cols = slice(half * 512, (half + 1) * 512)
                        w = w_ps.tile([P, 512], f32, tag="wps")
                        nc.tensor.matmul(
                            w[:], at_p[:, st * P:(st + 1) * P], wo_sb[:, cols],
                            start=True, stop=True,
                        )
                        idx = st * 2 + half
                        if idx % 8 < 3:
                            nc.scalar.activation(osb[:, st, cols], w[:], Act.Copy)
                        else:
                            nc.vector.tensor_copy(osb[:, st, cols], w[:])
                rows = slice(b_p * S + ib_p * 512, b_p * S + (ib_p + 1) * 512)
                nc.sync.dma_start(
                    out_d[rows, :].rearrange("(st p) h -> p st h", p=P), osb[:]
                )

            prev = None
            for b in range(B):
                for ib in range(4):
                    q0 = b * S + ib * 512
                    njt = 4 * (ib + 1)
                    po0 = o_ps.tile([65, 512], f32, tag="ops")
                    po1 = o_ps.tile([65, 512], f32, tag="ops")
                    for j in range(njt):
                        jt = b * SJT + j
                        js = slice(jt * P, (jt + 1) * P)
                        diag = j >= 4 * ib
                        jj = j - 4 * ib if diag else 0
                        lo = 128 * jj  # columns below lo are fully masked
                        qsl = slice(q0 + lo, q0 + 512)
                        s2 = s_ps.tile([P, 1024], f32, tag="sps")
                        nc.tensor.matmul(
                            s2[:, lo:512], kvT[0:64, js], qT[0:64, qsl],
                            start=True, stop=True,
                        )
                        nc.tensor.matmul(
                            s2[:, 512 + lo:1024], khi[64:128, js], qT[64:128, qsl],
                            start=True, stop=True,
                        )
                        p2 = p_pool.tile([P, 2, 512], adt, tag="ptile")
                        s2v = s2.rearrange("p (h c) -> p h c", h=2)
                        nc.scalar.activation(
                            p2[:, :, lo:512], s2v[:, :, lo:512], Act.Exp, scale=SCALE
                        )
                        if diag:
                            # staircase mask only matters in the 128-col band
                            # [lo, lo+128); all later columns are fully kept
                            hi = lo + 128
                            nc.vector.tensor_mul(
                                p2[:, :, lo:hi], p2[:, :, lo:hi],
                                mask_sb[:, jj, :, lo:hi],
                            )
                        nc.tensor.matmul(
                            po0[:, lo:512], vnat[:, jt, 0:65], p2[:, 0, lo:512],
                            start=(j == 0), stop=(j == njt - 1),
                            skip_group_check=True,
                        )
                        nc.tensor.matmul(
                            po1[:, lo:512], vnat[:, jt, 0:65], p2[:, 1, lo:512],
                            start=(j == 0), stop=(j == njt - 1),
                            skip_group_check=True,
                        )

                    # normalize: recip of ones-row (DVE), then broadcast via PE
                    r0 = r_pool.tile([1, 512], adt, tag="recip")
                    r1 = r_pool.tile([1, 512], adt, tag="recip")
                    with nc.allow_low_precision(reason="bf16 recip is enough"):
                        nc.vector.reciprocal(r0[:], po0[64:65, :])
                        nc.vector.reciprocal(r1[:], po1[64:65, :])

                    # deferred o_proj for the previous block fills the PE while
                    # the recip/broadcast chain of this block drains
                    if prev is not None:
                        emit_oproj(prev)

                    pb0 = w_ps.tile([64, 512], f32, tag="wps")
                    nc.tensor.matmul(pb0[:], ones_sb[:], r0[:], start=True, stop=True)
                    pb1 = w_ps.tile([64, 512], f32, tag="wps")
                    nc.tensor.matmul(pb1[:], ones_sb[:], r1[:], start=True, stop=True)
                    pbs0 = tmp_pool.tile([64, 512], f32, tag="pbs")
                    nc.scalar.activation(pbs0[:], pb0[:], Act.Copy)
                    pbs1 = tmp_pool.tile([64, 512], f32, tag="pbs")
                    nc.vector.tensor_copy(pbs1[:], pb1[:])
                    at = attn_pool.tile([P, 512], adt, tag="attn")
                    nc.vector.tensor_mul(at[0:64, :], po0[0:64, :], pbs0[:])
                    a1 = a1_pool.tile([64, 512], adt, tag="attn1")
                    nc.vector.tensor_mul(a1[:], po1[0:64, :], pbs1[:])
                    nc.sync.dma_start(at[64:128, :], a1[:])
                    prev = (at, b, ib)

            emit_oproj(prev)

    nc.compile()
    return nc


def _get_nc(mm_mode="bf16"):
    if mm_mode not in _NC_CACHE:
        _NC_CACHE[mm_mode] = _build_nc(mm_mode)
    return _NC_CACHE[mm_mode]


def make_in_maps(inputs, mm_mode="bf16"):
    """Host-side sharding/layout prep: returns the 8 per-core input dicts."""
    import ml_dtypes

    a_np = ml_dtypes.bfloat16
    hidden = np.asarray(inputs["hidden_states"], dtype=np.float32)
    pos = np.asarray(inputs["positions"])
    Wq = np.asarray(inputs["Wq"], dtype=np.float32)
    bq = np.asarray(inputs["bq"], dtype=np.float32)
    Wk = np.asarray(inputs["Wk"], dtype=np.float32)
    bk = np.asarray(inputs["bk"], dtype=np.float32)
    Wv = np.asarray(inputs["Wv"], dtype=np.float32)
    bv = np.asarray(inputs["bv"], dtype=np.float32)
    Wo = np.asarray(inputs["Wo"], dtype=np.float32)

    xT = np.ascontiguousarray(hidden.reshape(T, H).T)

    half = HD // 2  # 32
    inv = 1.0 / THETA ** (np.arange(half, dtype=np.float64) * 2.0 / HD)
    f = pos.astype(np.float64)[None, :] * inv[:, None]          # [32, S]
    cos32 = np.cos(f)
    sin32 = np.sin(f)
    pidx = np.arange(P) % half
    sgn = np.where(np.arange(P) % HD < half, -1.0, 1.0)
    cosT = np.ascontiguousarray(cos32[pidx].astype(np.float32))
    sinT = np.ascontiguousarray((sin32[pidx] * sgn[:, None]).astype(np.float32))

    m = np.arange(P)
    sig = np.where(m % HD < half, m + half, m - half)
    perm = np.zeros((P, P), np.float32)
    perm[sig, m] = 1.0
    ident = np.eye(P, dtype=np.float32)
    ones64 = np.ones((1, 64), np.float32)
    onescol = np.ones((P, 2 * (S // P)), np.float32)

    # causal masks for the 4 diagonal-band alignments: keep when within the
    # 512-query block the query index f >= key partition p + 128*jj
    fidx = np.arange(512)[None, :]
    pcol = np.arange(P)[:, None]
    masks = np.zeros((P, 4, 2, 512), np.float32)
    for jj in range(4):
        mk = (fidx >= pcol + 128 * jj).astype(np.float32)
        masks[:, jj, 0, :] = mk
        masks[:, jj, 1, :] = mk

    xTa = xT.astype(a_np)
    in_maps = []
    for c in range(NCORES):
        g = c // 2  # kv head for this core's 2 q heads
        wkv = np.ascontiguousarray(
            np.concatenate(
                [Wk[:, g * HD:(g + 1) * HD], Wv[:, g * HD:(g + 1) * HD]], axis=1
            )
        )
        bkv = np.ascontiguousarray(
            np.concatenate([bk[g * HD:(g + 1) * HD], bv[g * HD:(g + 1) * HD]])[:, None]
        )
        in_maps.append({
            "xT": xTa,
            "wq": np.ascontiguousarray(Wq[:, c * P:(c + 1) * P]).astype(a_np),
            "bq": np.ascontiguousarray(bq[c * P:(c + 1) * P][:, None]),
            "wkv": wkv.astype(a_np),
            "bkv": bkv,
            "wo": np.ascontiguousarray(Wo[c * P:(c + 1) * P, :]).astype(a_np),
            "cosT": cosT.astype(a_np),
            "sinT": sinT.astype(a_np),
            "perm": perm.astype(a_np),
            "ident": ident.astype(a_np),
            "ones64": ones64.astype(a_np),
            "onescol": onescol.astype(a_np),
            "masks": masks.astype(a_np),
        })
    return in_maps


def kernel(**inputs):
    global LAST_RESULT
    from concourse.bass_utils import run_bass_kernel_spmd

    mm_mode = os.environ.get("KERNEL_MM_MODE", "bf16")
    nc = _get_nc(mm_mode)
    in_maps = make_in_maps(inputs, mm_mode)
    res = run_bass_kernel_spmd(nc, in_maps, core_ids=list(range(NCORES)))
    LAST_RESULT = res
    out = res.results[0]["out"].astype(np.float32)
    for rr in res.results[1:]:
        out = out + rr["out"].astype(np.float32)
    return out.reshape(B, S, H)


# revision 16
# speedup vs baseline: 1.4210x; 1.1137x over previous
"""Trainium2 Bass kernel for MiMoAudio attention (GQA + neox RoPE + causal softmax + o_proj).

Strategy (tensor-parallel over heads, 8 cores):
  - Each core owns 2 of the 16 q heads (128 q channels) and the single kv head
    (64 channels) that those q heads attend to (GQA group).
  - Host pre-transposes hidden_states to xT [H, B*S] so every on-device matmul
    contracts over the partition dim with no on-device transposition of x.
  - All activations live "feature-on-partitions" (transposed domain):
      qT [128, T], kT/vT in kvT [128, T], scoresT [j, i], attnT [d', i].
    Softmax runs without max-subtraction (logits are O(10), fp32-exp safe);
    the denominator is obtained by an appended ones-row in the PV matmul.
  - o_proj row-slice per core produces a partial [T, H] output in bf16; host
    sums the 8 partials in fp32 (the TP all-reduce, done at unshard time).

Perf structure (vs the naive version):
  - Scores for both heads of a j-tile go into one 2-bank PSUM tile [128,1024]
    so a single Exp activation covers both heads (halves ACT instr overhead).
  - On diagonal-band j-tiles, the columns that causality fully masks are
    skipped in the scores matmul, the exp, and the PV matmul (column lo=128*jj
    onward only) — saves ~15% of exp volume and ~10% of attention PE work.
  - Residual staircase masking via a multiply with precomputed bf16 mask
    tiles, split between DVE and gpsimd (gpsimd cannot touch PSUM, so it only
    gets SBUF->SBUF work).
  - o_proj of block N is emitted after the scores of block N+1 (one-stage
    software pipeline) so the PE never waits on the normalize chain.
  - v-transposes are deferred to the end of phase A so the PE stream never
    stalls on the rope chain.
  - Output written as bf16 (halves output DMA); host sums in fp32.
"""

import os
import numpy as np

# Problem constants (hardcoded per contract; kernel.py must be self-contained).
B = 2
S = 2048
T = B * S          # 4096 flattened tokens
H = 1024           # hidden
HD = 64            # head dim
P = 128
NCORES = 8
THETA = 10000.0
SCALE = HD ** -0.5
NBLK = T // 512    # 8 token blocks of 512
HO = H // P        # 8 hidden chunks of 128
SJT = S // P       # 16 key tiles per batch

_NC_CACHE = {}
LAST_RESULT = None  # stash of the last BassKernelResults (for test harnesses)


def _ensure_ntff_hook():
    """Provide antenv.axon_hooks if the image lacks it, so BASS_TRACE=1
    profiling works under axon instead of crashing on import."""
    import sys
    import types

    try:
        import antenv.axon_hooks  # noqa: F401
        return
    except ImportError:
        pass
    mod = types.ModuleType("antenv.axon_hooks")
    mod._hook = None

    def set_axon_ntff_profile_hook(h):
        mod._hook = h

    def get_axon_ntff_profile_hook():
        return mod._hook

    mod.set_axon_ntff_profile_hook = set_axon_ntff_profile_hook
    mod.get_axon_ntff_profile_hook = get_axon_ntff_profile_hook
    sys.modules["antenv.axon_hooks"] = mod
    try:
        import antenv

        antenv.axon_hooks = mod
    except ImportError:
        pass
    try:
        from trn_agent_boot.trn_boot import _ntff_profile_via_ctypes

        hook = _ntff_profile_via_ctypes("/opt/axon/libaxon_pjrt.so")
        if hook is not None:
            mod.set_axon_ntff_profile_hook(hook)
    except Exception:
        pass


_ensure_ntff_hook()


def _build_nc(mm_mode="bf16"):
    import concourse.bass as bass
    import concourse.mybir as mybir
    import concourse.tile as tile
    from concourse import bacc

    assert mm_mode == "bf16", "only bf16 supported"
    f32 = mybir.dt.float32
    adt = mybir.dt.bfloat16
    Act = mybir.ActivationFunctionType

    nc = bacc.Bacc(None, target_bir_lowering=False, debug=False)

    # --- DRAM I/O ------------------------------------------------------------
    xT_d = nc.dram_tensor("xT", [H, T], adt, kind="ExternalInput")
    wq_d = nc.dram_tensor("wq", [H, P], adt, kind="ExternalInput")
    bq_d = nc.dram_tensor("bq", [P, 1], f32, kind="ExternalInput")
    wkv_d = nc.dram_tensor("wkv", [H, P], adt, kind="ExternalInput")
    bkv_d = nc.dram_tensor("bkv", [P, 1], f32, kind="ExternalInput")
    wo_d = nc.dram_tensor("wo", [P, H], adt, kind="ExternalInput")
    cos_d = nc.dram_tensor("cosT", [P, S], adt, kind="ExternalInput")
    sin_d = nc.dram_tensor("sinT", [P, S], adt, kind="ExternalInput")
    perm_d = nc.dram_tensor("perm", [P, P], adt, kind="ExternalInput")
    id_d = nc.dram_tensor("ident", [P, P], adt, kind="ExternalInput")
    ones_d = nc.dram_tensor("ones64", [1, 64], adt, kind="ExternalInput")
    onescol_d = nc.dram_tensor("onescol", [P, 2 * SJT], adt, kind="ExternalInput")
    mask_d = nc.dram_tensor("masks", [P, 4, 2, 512], adt, kind="ExternalInput")
    out_d = nc.dram_tensor("out", [T, H], adt, kind="ExternalOutput")

    with tile.TileContext(nc) as tc:
        with (
            tc.tile_pool(name="const", bufs=1) as cpool,
            tc.tile_pool(name="persist", bufs=1) as ppool,
            tc.tile_pool(name="xt", bufs=3) as xt_pool,
            tc.tile_pool(name="ptile", bufs=3) as p_pool,
            tc.tile_pool(name="attn", bufs=2) as attn_pool,
            tc.tile_pool(name="attn1", bufs=2) as a1_pool,
            tc.tile_pool(name="recip", bufs=4) as r_pool,
            tc.tile_pool(name="tmp", bufs=4) as tmp_pool,
            tc.tile_pool(name="osb", bufs=2) as osb_pool,
            tc.tile_pool(name="sps", bufs=2, space="PSUM") as s_ps,
            tc.tile_pool(name="ops", bufs=2, space="PSUM") as o_ps,
            tc.tile_pool(name="wps", bufs=2, space="PSUM") as w_ps,
        ):
            # --- constant loads ---------------------------------------------
            # Order matters: the first projection only needs wq/bq and xt of
            # blk 0, so those DMAs go first; bulky constants follow.
            wq_sb = cpool.tile([P, HO, P], adt)
            nc.sync.dma_start(wq_sb[:], wq_d[:].rearrange("(o p) m -> p o m", p=P))
            bq_sb = cpool.tile([P, 1], f32)
            nc.sync.dma_start(bq_sb[:], bq_d[:])

            xT_r = xT_d[:].rearrange("(o p) t -> p o t", p=P)
            xts = []
            for blk in range(3):
                xt = xt_pool.tile([P, HO, 512], adt, tag="xt", name=f"xt{blk}")
                nc.sync.dma_start(xt[:], xT_r[:, :, blk * 512:(blk + 1) * 512])
                xts.append(xt)

            wkv_sb = cpool.tile([P, HO, P], adt)
            nc.sync.dma_start(wkv_sb[:], wkv_d[:].rearrange("(o p) m -> p o m", p=P))
            bkv_sb = cpool.tile([P, 1], f32)
            nc.sync.dma_start(bkv_sb[:], bkv_d[:])
            perm_sb = cpool.tile([P, P], adt)
            nc.sync.dma_start(perm_sb[:], perm_d[:])
            cos_sb = cpool.tile([P, S], adt)
            nc.sync.dma_start(cos_sb[:], cos_d[:])
            sin_sb = cpool.tile([P, S], adt)
            nc.sync.dma_start(sin_sb[:], sin_d[:])
            id_sb = cpool.tile([P, P], adt)
            nc.sync.dma_start(id_sb[:], id_d[:])
            ones_sb = cpool.tile([1, 64], adt)
            nc.sync.dma_start(ones_sb[:], ones_d[:])
            wo_sb = cpool.tile([P, H], adt)
            nc.sync.dma_start(wo_sb[:], wo_d[:])
            mask_sb = cpool.tile([P, 4, 2, 512], adt)
            nc.sync.dma_start(mask_sb[:], mask_d[:])

            # --- persistent activation tiles --------------------------------
            qT = ppool.tile([P, T], adt)        # 2 q heads stacked (rows h*64+d)
            kvT = ppool.tile([P, T], adt)       # k rows 0:64, v rows 64:128
            khi = ppool.tile([P, T], adt)       # k duplicated at rows 64:128
            vnat = ppool.tile([P, 2 * SJT, 72], adt)  # v natural [j, d] + ones col

            # ones column for the PV denominator row
            nc.sync.dma_start(
                vnat[:, :, 64:65], onescol_d[:].rearrange("p (j o) -> p j o", o=1)
            )

            # --- phase A: QKV projection + RoPE -----------------------------
            for blk in range(NBLK):
                tb = slice(blk * 512, (blk + 1) * 512)
                if blk < 3:
                    xt = xts[blk]
                else:
                    xt = xt_pool.tile([P, HO, 512], adt, tag="xt")
                    nc.sync.dma_start(xt[:], xT_r[:, :, tb])

                qkv = s_ps.tile([P, 1024], f32, tag="sps")
                for o in range(HO):
                    nc.tensor.matmul(
                        qkv[:, 0:512], wq_sb[:, o, :], xt[:, o, :],
                        start=(o == 0), stop=(o == HO - 1),
                    )
                for o in range(HO):
                    nc.tensor.matmul(
                        qkv[:, 512:1024], wkv_sb[:, o, :], xt[:, o, :],
                        start=(o == 0), stop=(o == HO - 1),
                    )
                nc.vector.tensor_scalar_add(qT[:, tb], qkv[:, 0:512], bq_sb[:])
                nc.vector.tensor_scalar_add(kvT[:, tb], qkv[:, 512:1024], bkv_sb[:])

                sc = (blk * 512) % S
                ss = slice(sc, sc + 512)

                # RoPE(q): partner = perm @ qT (partition rotate by 32 within 64)
                pr = w_ps.tile([P, 512], f32, tag="wps")
                nc.tensor.matmul(pr[:], perm_sb[:], qT[:, tb], start=True, stop=True)
                tmp = tmp_pool.tile([P, 512], adt, tag="ropetmp")
                nc.vector.tensor_mul(tmp[:], pr[:], sin_sb[:, ss])
                nc.gpsimd.tensor_mul(qT[:, tb], qT[:, tb], cos_sb[:, ss])
                nc.gpsimd.tensor_add(qT[:, tb], qT[:, tb], tmp[:])

                # RoPE(k): rows 0:64 only
                prk = w_ps.tile([P, 512], f32, tag="wps")
                nc.tensor.matmul(
                    prk[0:64, :], perm_sb[0:64, 0:64], kvT[0:64, tb],
                    start=True, stop=True,
                )
                tmpk = tmp_pool.tile([P, 512], adt, tag="ropetmp")
                nc.vector.tensor_mul(tmpk[0:64, :], prk[0:64, :], sin_sb[0:64, ss])
                nc.gpsimd.tensor_mul(kvT[0:64, tb], kvT[0:64, tb], cos_sb[0:64, ss])
                nc.gpsimd.tensor_add(kvT[0:64, tb], kvT[0:64, tb], tmpk[0:64, :])

                # duplicate rope'd k at rows 64:128 (for head-1 row-group matmuls)
                nc.sync.dma_start(khi[64:128, tb], kvT[0:64, tb])

            # --- phase A2: v natural layout [j, d] per 128-key tile ---------
            for jt in range(2 * SJT):
                tp = w_ps.tile([P, 512], adt, tag="wps")
                nc.tensor.transpose(
                    tp[:, 0:64], kvT[64:128, jt * P:(jt + 1) * P],
                    id_sb[64:128, 64:128],
                )
                nc.vector.tensor_copy(vnat[:, jt, 0:64], tp[:, 0:64])

            # --- phase B: attention + (deferred) o_proj ---------------------
            def emit_oproj(prev):
                at_p, b_p, ib_p = prev
                osb = osb_pool.tile([P, 4, H], adt, tag="osb")
                for st in range(4):
                    for half in range(2):
                        cols = slice(half * 512, (half + 1) * 512)
                        w = w_ps.tile([P, 512], f32, tag="wps")
                        nc.tensor.matmul(
                            w[:], at_p[:, st * P:(st + 1) * P], wo_sb[:, cols],
                            start=True, stop=True,
                        )
                        nc.vector.tensor_copy(osb[:, st, cols], w[:])
                rows = slice(b_p * S + ib_p * 512, b_p * S + (ib_p + 1) * 512)
                nc.sync.dma_start(
                    out_d[rows, :].rearrange("(st p) h -> p st h", p=P), osb[:]
                )

            prev = None
            for b in range(B):
                for ib in range(4):
                    q0 = b * S + ib * 512
                    njt = 4 * (ib + 1)
                    po0 = o_ps.tile([65, 512], f32, tag="ops")
                    po1 = o_ps.tile([65, 512], f32, tag="ops")
                    for j in range(njt):
                        jt = b * SJT + j
                        js = slice(jt * P, (jt + 1) * P)
                        diag = j >= 4 * ib
                        jj = j - 4 * ib if diag else 0
                        lo = 128 * jj  # columns below lo are fully masked
                        qsl = slice(q0 + lo, q0 + 512)
                        s2 = s_ps.tile([P, 1024], f32, tag="sps")
                        nc.tensor.matmul(
                            s2[:, lo:512], kvT[0:64, js], qT[0:64, qsl],
                            start=True, stop=True,
                        )
                        nc.tensor.matmul(
                            s2[:, 512 + lo:1024], khi[64:128, js], qT[64:128, qsl],
                            start=True, stop=True,
                        )
                        p2 = p_pool.tile([P, 2, 512], adt, tag="ptile")
                        s2v = s2.rearrange("p (h c) -> p h c", h=2)
                        nc.scalar.activation(
                            p2[:, :, lo:512], s2v[:, :, lo:512], Act.Exp, scale=SCALE
                        )
                        if diag:
                            # staircase mask only matters in the 128-col band
                            # [lo, lo+128); all later columns are fully kept
                            hi = lo + 128
                            nc.gpsimd.tensor_mul(
                                p2[:, :, lo:hi], p2[:, :, lo:hi],
                                mask_sb[:, jj, :, lo:hi],
                            )
                        nc.tensor.matmul(
                            po0[:, lo:512], vnat[:, jt, 0:65], p2[:, 0, lo:512],
                            start=(j == 0), stop=(j == njt - 1),
                            skip_group_check=True,
                        )
                        nc.tensor.matmul(
                            po1[:, lo:512], vnat[:, jt, 0:65], p2[:, 1, lo:512],
                            start=(j == 0), stop=(j == njt - 1),
                            skip_group_check=True,
                        )

                    # normalize: 1/Z = exp(-ln(Z)) on ACT — both functions
                    # live in the same activation table (no reload), and ACT
                    # row ops are ~600ns vs 3.3us for vector.reciprocal on a
                    # [1,512] single-partition row.
                    lnz0 = r_pool.tile([1, 512], f32, tag="lnz")
                    lnz1 = r_pool.tile([1, 512], f32, tag="lnz")
                    nc.scalar.activation(lnz0[:], po0[64:65, :], Act.Ln)
                    nc.scalar.activation(lnz1[:], po1[64:65, :], Act.Ln)
                    rv0 = r_pool.tile([1, 512], adt, tag="recip")
                    rv1 = r_pool.tile([1, 512], adt, tag="recip")
                    nc.scalar.activation(rv0[:], lnz0[:], Act.Exp, scale=-1.0)
                    nc.scalar.activation(rv1[:], lnz1[:], Act.Exp, scale=-1.0)

                    # deferred o_proj for the previous block fills the PE while
                    # the normalize chain of this block drains
                    if prev is not None:
                        emit_oproj(prev)

                    pb0 = w_ps.tile([64, 512], f32, tag="wps")
                    nc.tensor.matmul(pb0[:], ones_sb[:], rv0[:], start=True, stop=True)
                    pb1 = w_ps.tile([64, 512], f32, tag="wps")
                    nc.tensor.matmul(pb1[:], ones_sb[:], rv1[:], start=True, stop=True)
                    pbs0 = tmp_pool.tile([64, 512], f32, tag="pbs")
                    nc.vector.tensor_copy(pbs0[:], pb0[:])
                    pbs1 = tmp_pool.tile([64, 512], f32, tag="pbs")
                    nc.vector.tensor_copy(pbs1[:], pb1[:])
                    at = attn_pool.tile([P, 512], adt, tag="attn")
                    nc.vector.tensor_mul(at[0:64, :], po0[0:64, :], pbs0[:])
                    a1 = a1_pool.tile([64, 512], adt, tag="attn1")
                    nc.vector.tensor_mul(a1[:], po1[0:64, :], pbs1[:])
                    nc.sync.dma_start(at[64:128, :], a1[:])
                    prev = (at, b, ib)

            emit_oproj(prev)

    nc.compile()
    return nc


def _get_nc(mm_mode="bf16"):
    if mm_mode not in _NC_CACHE:
        _NC_CACHE[mm_mode] = _build_nc(mm_mode)
    return _NC_CACHE[mm_mode]


def make_in_maps(inputs, mm_mode="bf16"):
    """Host-side sharding/layout prep: returns the 8 per-core input dicts."""
    import ml_dtypes

    a_np = ml_dtypes.bfloat16
    hidden = np.asarray(inputs["hidden_states"], dtype=np.float32)
    pos = np.asarray(inputs["positions"])
    Wq = np.asarray(inputs["Wq"], dtype=np.float32)
    bq = np.asarray(inputs["bq"], dtype=np.float32)
    Wk = np.asarray(inputs["Wk"], dtype=np.float32)
    bk = np.asarray(inputs["bk"], dtype=np.float32)
    Wv = np.asarray(inputs["Wv"], dtype=np.float32)
    bv = np.asarray(inputs["bv"], dtype=np.float32)
    Wo = np.asarray(inputs["Wo"], dtype=np.float32)

    xT = np.ascontiguousarray(hidden.reshape(T, H).T)

    half = HD // 2  # 32
    inv = 1.0 / THETA ** (np.arange(half, dtype=np.float64) * 2.0 / HD)
    f = pos.astype(np.float64)[None, :] * inv[:, None]          # [32, S]
    cos32 = np.cos(f)
    sin32 = np.sin(f)
    pidx = np.arange(P) % half
    sgn = np.where(np.arange(P) % HD < half, -1.0, 1.0)
    cosT = np.ascontiguousarray(cos32[pidx].astype(np.float32))
    sinT = np.ascontiguousarray((sin32[pidx] * sgn[:, None]).astype(np.float32))

    m = np.arange(P)
    sig = np.where(m % HD < half, m + half, m - half)
    perm = np.zeros((P, P), np.float32)
    perm[sig, m] = 1.0
    ident = np.eye(P, dtype=np.float32)
    ones64 = np.ones((1, 64), np.float32)
    onescol = np.ones((P, 2 * (S // P)), np.float32)

    # causal masks for the 4 diagonal-band alignments: keep when within the
    # 512-query block the query index f >= key partition p + 128*jj
    fidx = np.arange(512)[None, :]
    pcol = np.arange(P)[:, None]
    masks = np.zeros((P, 4, 2, 512), np.float32)
    for jj in range(4):
        mk = (fidx >= pcol + 128 * jj).astype(np.float32)
        masks[:, jj, 0, :] = mk
        masks[:, jj, 1, :] = mk

    xTa = xT.astype(a_np)
    in_maps = []
    for c in range(NCORES):
        g = c // 2  # kv head for this core's 2 q heads
        wkv = np.ascontiguousarray(
            np.concatenate(
                [Wk[:, g * HD:(g + 1) * HD], Wv[:, g * HD:(g + 1) * HD]], axis=1
            )
        )
        bkv = np.ascontiguousarray(
            np.concatenate([bk[g * HD:(g + 1) * HD], bv[g * HD:(g + 1) * HD]])[:, None]
        )
        in_maps.append({
            "xT": xTa,
            "wq": np.ascontiguousarray(Wq[:, c * P:(c + 1) * P]).astype(a_np),
            "bq": np.ascontiguousarray(bq[c * P:(c + 1) * P][:, None]),
            "wkv": wkv.astype(a_np),
            "bkv": bkv,
            "wo": np.ascontiguousarray(Wo[c * P:(c + 1) * P, :]).astype(a_np),
            "cosT": cosT.astype(a_np),
            "sinT": sinT.astype(a_np),
            "perm": perm.astype(a_np),
            "ident": ident.astype(a_np),
            "ones64": ones64.astype(a_np),
            "onescol": onescol.astype(a_np),
            "masks": masks.astype(a_np),
        })
    return in_maps


def kernel(**inputs):
    global LAST_RESULT
    from concourse.bass_utils import run_bass_kernel_spmd

    mm_mode = os.environ.get("KERNEL_MM_MODE", "bf16")
    nc = _get_nc(mm_mode)
    in_maps = make_in_maps(inputs, mm_mode)
    res = run_bass_kernel_spmd(nc, in_maps, core_ids=list(range(NCORES)))
    LAST_RESULT = res
    out = res.results[0]["out"].astype(np.float32)
    for rr in res.results[1:]:
        out = out + rr["out"].astype(np.float32)
    return out.reshape(B, S, H)
